# revision 28
# baseline (speedup 1.0000x reference)
"""LLaMA causal self-attention (GQA) on 8 Trainium2 NeuronCores.

Sharding: 2-way data-parallel over batch x 4-way tensor-parallel over KV
groups. Core cid handles batch b=cid//4 and KV group g=cid%4 (q heads
4g..4g+3, kv head g). Each core computes a partial output y_partial =
att_heads @ Wo_rows; the host sums the 4 partials per batch and adds bo.

Per-core pipeline (all layouts chosen so matmul contraction is on the
partition dim and softmax needs no transposes):
  P1: QKV projection (x^T chunks stationary). RMSNorm stats read the
      PSUM accumulator directly on DVE; rstd via ACT Sqrt + DVE recip +
      one fused-Newton step. RoPE operates on a de-interleaved head
      layout (even dims | odd dims per head, weights permuted host-side)
      so every DVE/Pool op runs on contiguous [128,5,64] APs covering
      all 4 q heads + k in one instruction. PE-transpose q/k to [d, t],
      copy-out on ACT, 3-deep software pipeline.
  P2: attention computed transposed: scoresT[k, q] = kT_j^T @ qT chunks,
      additive causal mask on diagonal blocks, exp without max
      subtraction, softmax denominators via an all-ones stationary
      matmul, PV accumulated in PSUM over j. Emission keeps the PE one
      sc-matmul ahead of the exp it waits on.
  P3: output projection from attT chunks, double-buffered PSUM so the
      PE never stalls (keeps the 2.4 GHz p-state), PSUM -> SBUF copies
      alternate ACT/DVE, DMA out per 128-token block.

Phases are interleaved at emission time: the last three transpose
batches weave between the first attention groups, and the 16 output-
projection blocks weave between later attention groups (each block is
emitted as soon as the attT columns it reads are final), so the PE
queue never drains across phase boundaries.
"""

import os
from contextlib import ExitStack

import numpy as np

B, T, C = 2, 2048, 2048
H, KV = 16, 4
D = 128
HQ = H // KV        # q heads per core = 4
TB = T // 128       # 16
CB = C // 128       # 16
EPS = 1e-5
SCALE = float(np.float32(1.0) / np.sqrt(np.float32(D)))

_CACHE = {}


def _build(dt_name, with_bias=True):
    import concourse.bass as bass
    import concourse.bacc as bacc
    from concourse import mybir
    from concourse.tile import TileContext

    DT = getattr(mybir.dt, dt_name)
    F32 = mybir.dt.float32
    AF = mybir.ActivationFunctionType
    ALU = mybir.AluOpType

    nc = bacc.Bacc(None, target_bir_lowering=False)
    xt = nc.dram_tensor("xt", [TB, 128, CB * 128], DT, kind="ExternalInput")
    wqkv = nc.dram_tensor("wqkv", [128, CB * 768], DT, kind="ExternalInput")
    bqkv = nc.dram_tensor("bqkv", [1, 768], DT, kind="ExternalInput")
    trig = nc.dram_tensor("trig", [TB, 128, 4 * 5 * 64], DT, kind="ExternalInput")
    cst = nc.dram_tensor("cst", [4, 128, 128], DT, kind="ExternalInput")
    wo = nc.dram_tensor("wo", [128, HQ * C], DT, kind="ExternalInput")
    y = nc.dram_tensor("y", [T, C], DT, kind="ExternalOutput")

    with TileContext(nc) as tc, ExitStack() as ctx:
        persist = ctx.enter_context(tc.tile_pool(name="persist", bufs=1))
        # [d, seg, t]: segs 0..3 = q heads, seg 4 = k
        qkT = persist.tile([128, 5, T], DT)
        vbuf = persist.tile([128, TB, 128], DT)   # [t-in-block, j, d]
        attT = persist.tile([128, HQ, T], DT)     # [d, head, t]
        wqkv_sb = persist.tile([128, CB, 768], DT)
        wo_sb = persist.tile([128, HQ, C], DT)
        ones = persist.tile([128, 128], DT)
        ident = persist.tile([128, 128], DT)
        maskt_sb = persist.tile([128, 128], DT)
        bq_sb = persist.tile([1, 768], DT)
        eps_sb = persist.tile([128, 1], F32)

        # ---------------- P1: QKV + RMSNorm + RoPE + transpose ----------
        # P1 pools live in their own stack so their PSUM banks free up
        # before the attention pools are created. p1r (qr tiles) must
        # outlive the last transpose batches woven into the qq0 row.
        p1r = ctx.enter_context(tc.tile_pool(name="p1r", bufs=5))
        p1_ctx = ExitStack()
        p1x = p1_ctx.enter_context(tc.tile_pool(name="p1x", bufs=3))
        p1t = p1_ctx.enter_context(tc.tile_pool(name="p1t", bufs=3))
        p1s = p1_ctx.enter_context(tc.tile_pool(name="p1s", bufs=3))
        p1q = p1_ctx.enter_context(tc.tile_pool(name="p1q", bufs=3))
        p1ps = p1_ctx.enter_context(tc.tile_pool(name="p1ps", bufs=2, space="PSUM"))
        p1tp = p1_ctx.enter_context(tc.tile_pool(name="p1tp", bufs=2, space="PSUM"))

        # startup: stream xt0 in quarters so the first QKV matmuls can
        # begin after ~1/4 of the tile; weave wqkv chunks in between.
        xt0 = p1x.tile([128, CB, 128], DT, tag="xt")
        nc.sync.dma_start(out=xt0[:, 0:4, :], in_=xt[0, :, 0:512])
        nc.sync.dma_start(out=wqkv_sb[:, 0:2, :], in_=wqkv[:, 0:1536])
        nc.sync.dma_start(out=xt0[:, 4:8, :], in_=xt[0, :, 512:1024])
        nc.sync.dma_start(out=wqkv_sb[:, 2:4, :], in_=wqkv[:, 1536:3072])
        nc.sync.dma_start(out=xt0[:, 8:12, :], in_=xt[0, :, 1024:1536])
        nc.sync.dma_start(out=wqkv_sb[:, 4:6, :], in_=wqkv[:, 3072:4608])
        nc.sync.dma_start(out=xt0[:, 12:16, :], in_=xt[0, :, 1536:2048])
        trig0 = p1t.tile([128, 4, 5, 64], DT, tag="trig")
        nc.sync.dma_start(out=trig0, in_=trig[0])
        for wc in range(3, 8):
            nc.sync.dma_start(out=wqkv_sb[:, wc * 2:(wc + 1) * 2, :],
                              in_=wqkv[:, wc * 1536:(wc + 1) * 1536])
        nc.sync.dma_start(out=ones, in_=cst[0])
        nc.sync.dma_start(out=ident, in_=cst[1])
        nc.sync.dma_start(out=maskt_sb, in_=cst[2])
        if with_bias:
            nc.sync.dma_start(out=bq_sb, in_=bqkv[:, :])
        nc.gpsimd.memset(eps_sb, EPS)

        pend = []

        def transposes(qr_prev, ttp, pool):
            # all 5 transposes land in ONE PSUM tile (disjoint 128-col
            # slices), then one wide copy to qkT on ACT
            tp = pool.tile([128, 5, 128], DT, tag="tp")
            for s in range(5):
                nc.tensor.transpose(tp[:, s, :],
                                    qr_prev[:, s * 128:(s + 1) * 128],
                                    ident)
            nc.scalar.activation(
                out=qkT[:, 0:5, ttp * 128:(ttp + 1) * 128], in_=tp,
                func=AF.Copy, scale=1.0, bias=0.0)

        for tt in range(TB):
            if tt == 0:
                xtall, trig_sb = xt0, trig0
            else:
                xtall = p1x.tile([128, CB, 128], DT, tag="xt")
                nc.sync.dma_start(out=xtall, in_=xt[tt])
                trig_sb = p1t.tile([128, 4, 5, 64], DT, tag="trig")
                nc.sync.dma_start(out=trig_sb, in_=trig[tt])
            if tt == 2:
                # wo needed only in P3; keep it off the early DMA path
                nc.sync.dma_start(out=wo_sb, in_=wo[:, :])

            qkv_ps = p1ps.tile([128, 768], F32, tag="qkv")
            last = CB - 1
            for cc in range(CB):
                stop_here = (not with_bias) and cc == last
                nc.tensor.matmul(qkv_ps[:, 0:512], xtall[:, cc, :],
                                 wqkv_sb[:, cc, 0:512],
                                 start=(cc == 0), stop=stop_here)
                nc.tensor.matmul(qkv_ps[:, 512:768], xtall[:, cc, :],
                                 wqkv_sb[:, cc, 512:768],
                                 start=(cc == 0), stop=stop_here)
            if with_bias:
                nc.tensor.matmul(qkv_ps[:, 0:512], ones[0:1, :],
                                 bq_sb[0:1, 0:512], start=False, stop=True)
                nc.tensor.matmul(qkv_ps[:, 512:768], ones[0:1, :],
                                 bq_sb[0:1, 512:768], start=False,
                                 stop=True)

            # software pipeline: transpose qr from THREE tiles back
            if len(pend) == 3:
                transposes(*pend.pop(0), p1tp)

            # RMSNorm stats: squares on ACT (single PSUM read), segmented
            # reduce + rstd + normalize on DVE -- except for the LAST
            # three tts, where the chain runs on ACT/Pool instead so the
            # DVE queue is empty when the first attention groups need
            # their causal-mask adds (DVE is the only engine that can
            # add the mask into the PSUM score tiles).
            # tt14/15 chains run on Pool: their transposes are woven late
            # into the second attention row, and keeping these chains off
            # DVE frees it for the first rows' causal-mask adds.
            tail = tt >= TB - 2
            ve = nc.gpsimd if tail else nc.vector
            sqs = p1s.tile([128, 640], F32, tag="sqs")
            ssq = p1s.tile([128, 8], F32, tag="ssq")
            nc.scalar.activation(out=sqs, in_=qkv_ps[:, 0:640],
                                 func=AF.Square, scale=1.0, bias=0.0)
            nc.vector.tensor_reduce(
                out=ssq[:, 0:5],
                in_=sqs.rearrange("p (s d) -> p s d", s=5, d=128),
                axis=mybir.AxisListType.X, op=ALU.add)
            # rstd = rsqrt(ssq/D + EPS): ACT Sqrt + DVE recip, then one
            # fused Newton step rstd = r0 * (1.5 - (0.5/D)*ssq*r0^2)
            sq5 = p1s.tile([128, 8], F32, tag="sq5")
            nc.scalar.activation(out=sq5[:, 0:5], in_=ssq[:, 0:5],
                                 func=AF.Sqrt, scale=1.0 / D,
                                 bias=eps_sb[:, 0:1])
            rstd = p1s.tile([128, 8], F32, tag="rstd")
            nc.vector.reciprocal_approx_fast(out=rstd[:, 0:5], in_=sq5[:, 0:5])

            # normalize q+k in ONE op: rstd broadcast over each seg.
            # Tail tts: ACT copies PSUM->SBUF first, Pool normalizes.
            qn = p1q.tile([128, 640], F32, tag="qn")
            rstd_b = bass.AP(tensor=rstd.tensor, offset=rstd.offset,
                             ap=[list(rstd.ap[0]), [1, 5], [0, 128]])
            if tail:
                qs = p1q.tile([128, 640], F32, tag="qs")
                nc.scalar.activation(out=qs, in_=qkv_ps[:, 0:640],
                                     func=AF.Copy, scale=1.0, bias=0.0)
                qsrc = qs
            else:
                qsrc = qkv_ps[:, 0:640]
            ve.tensor_mul(
                qn.rearrange("p (s d) -> p s d", s=5, d=128),
                qsrc.rearrange("p (s d) -> p s d", s=5, d=128),
                rstd_b)
            # v copy on ACT (PSUM -> SBUF bf16)
            nc.scalar.activation(out=vbuf[:, tt, :], in_=qkv_ps[:, 640:768],
                                 func=AF.Copy, scale=1.0, bias=0.0)

            # RoPE on de-interleaved layout: evens at [s*128, s*128+64),
            # odds at +64. One [128,5,64] op covers all 4 q heads + k.
            qr = p1r.tile([128, 640], DT, tag="qr")
            qn3 = qn.rearrange("p (s d) -> p s d", s=5, d=128)
            qr3 = qr.rearrange("p (s d) -> p s d", s=5, d=128)
            qe, qo = qn3[:, :, 0:64], qn3[:, :, 64:128]
            re, ro = qr3[:, :, 0:64], qr3[:, :, 64:128]
            ce, so = trig_sb[:, 0], trig_sb[:, 1]
            se, co = trig_sb[:, 2], trig_sb[:, 3]
            ta = p1s.tile([128, 5, 64], F32, tag="ra")
            tb = p1s.tile([128, 5, 64], F32, tag="rb")
            ve.tensor_mul(ta, qe, ce)
            nc.gpsimd.tensor_mul(tb, qo, so)
            ve.tensor_sub(re, ta, tb)
            tc_ = p1s.tile([128, 5, 64], F32, tag="rc")
            td = p1s.tile([128, 5, 64], F32, tag="rd")
            nc.gpsimd.tensor_mul(tc_, qe, se)
            ve.tensor_mul(td, qo, co)
            nc.gpsimd.tensor_add(ro, tc_, td)
            pend.append((qr, tt))

        # ------- P2+P3 interleaved: attention + output projection -------
        # PSUM budget during attn: sc 2 + outT 2 + sums 2 + (tpB 1 then
        # y 2) = 7-8 banks.
        p1_ctx.close()
        p2 = ctx.enter_context(tc.tile_pool(name="p2", bufs=6))
        p2a = ctx.enter_context(tc.tile_pool(name="p2a", bufs=6))
        p2v = ctx.enter_context(tc.tile_pool(name="p2v", bufs=2))
        p2o = ctx.enter_context(tc.tile_pool(name="p2o", bufs=2, space="PSUM"))
        p2s = ctx.enter_context(tc.tile_pool(name="p2s", bufs=2, space="PSUM"))
        p2sc = ctx.enter_context(tc.tile_pool(name="p2sc", bufs=2, space="PSUM"))
        tpB_ctx = ExitStack()
        p1tpB = tpB_ctx.enter_context(
            tc.tile_pool(name="p1tpB", bufs=1, space="PSUM"))

        def attn_head(h, qq):
            """Emit one attention group. Yields once after the first two
            sc matmuls so callers can weave PE filler work in.

            Softmax denominators: pT tiles accumulate on DVE (even j) and
            Pool (odd j) into SBUF fp32; one bf16 merge + a single
            all-ones matmul per group replaces the per-j PE sums matmuls.
            """
            q0 = qq * 512
            jmax = qq * 4 + 3
            outT = p2o.tile([128, 512], F32, tag="outT")
            accs = {}
            scs = {}

            def acc_update(j, pT, ob):
                # qq0 groups are tiny; keep them on DVE alone
                eng, key = ((nc.vector, "e") if (qq == 0 or j % 2 == 0)
                            else (nc.gpsimd, "o"))
                if key not in accs:
                    acc = p2a.tile([128, 512], F32, tag="acc" + key)
                    accs[key] = acc
                    eng.tensor_copy(acc, pT)      # j0/j1 are full-width
                else:
                    acc = accs[key]
                    eng.tensor_add(acc[:, ob:512], acc[:, ob:512],
                                   pT[:, ob:512])

            def emit_sc(j):
                qlo = max(q0, j * 128)
                ob = qlo - q0
                sc = p2sc.tile([128, 512], F32, tag="sc")
                nc.tensor.matmul(sc[:, ob:512], qkT[:, 4, j * 128:(j + 1) * 128],
                                 qkT[:, h, qlo:q0 + 512],
                                 start=True, stop=True)
                if j * 128 >= q0:  # diagonal: causal mask
                    nc.vector.tensor_add(sc[:, ob:ob + 128],
                                         sc[:, ob:ob + 128], maskt_sb)
                scs[j] = (sc, ob)

            emit_sc(0)
            emit_sc(1)
            yield
            for j in range(jmax + 1):
                sc, ob = scs.pop(j)
                pT = p2.tile([128, 512], DT, tag="pT")
                nc.scalar.activation(out=pT[:, ob:512], in_=sc[:, ob:512],
                                     func=AF.Exp, scale=SCALE)
                nc.tensor.matmul(outT[:, ob:512], vbuf[:, j, :],
                                 pT[:, ob:512],
                                 start=(j == 0), stop=(j == jmax),
                                 skip_group_check=True)
                acc_update(j, pT, ob)
                if j + 2 <= jmax:
                    emit_sc(j + 2)

            accbf = p2a.tile([128, 512], DT, tag="accbf")
            if "o" in accs:
                nc.gpsimd.tensor_add(accbf, accs["e"], accs["o"])
            else:
                nc.gpsimd.tensor_copy(accbf, accs["e"])

            def finalize():
                sums = p2s.tile([128, 512], F32, tag="sums")
                nc.tensor.matmul(sums, ones, accbf, start=True, stop=True)
                inv = p2v.tile([128, 512], F32, tag="inv")
                nc.vector.reciprocal_approx_fast(out=inv, in_=sums)
                nc.vector.tensor_mul(attT[:, h, q0:q0 + 512], outT, inv)

            pending_fin.append(finalize)

        p3_ctx = ExitStack()
        p3sb = None
        p3ps = None

        def p3_block(tt):
            nonlocal p3sb, p3ps
            if p3ps is None:
                p3sb = p3_ctx.enter_context(tc.tile_pool(name="p3sb", bufs=2))
                p3ps = p3_ctx.enter_context(
                    tc.tile_pool(name="p3ps", bufs=2, space="PSUM"))
            y_sb = p3sb.tile([128, 2048], DT, tag="ysb")
            for c4 in range(4):
                y_ps = p3ps.tile([128, 512], F32, tag="y")
                for hh in range(HQ):
                    nc.tensor.matmul(y_ps,
                                     attT[:, hh, tt * 128:(tt + 1) * 128],
                                     wo_sb[:, hh, c4 * 512:(c4 + 1) * 512],
                                     start=(hh == 0), stop=(hh == HQ - 1))
                dst = y_sb[:, c4 * 512:(c4 + 1) * 512]
                if c4 % 2 == 0:
                    nc.scalar.activation(out=dst, in_=y_ps, func=AF.Copy,
                                         scale=1.0, bias=0.0)
                else:
                    nc.vector.tensor_copy(dst, y_ps)
            nc.sync.dma_start(out=y[tt * 128:(tt + 1) * 128, :], in_=y_sb)

        # Row order [qq2, qq1, qq0, qq3]: the long, DVE-light qq2 row
        # runs first while DVE drains the P1 tail chains (the early rows
        # are short and DVE-heavy: masks + recip + attT-normalize nearly
        # saturate DVE). The last three transpose batches weave into the
        # qq1 row (their outputs are only read by the qq3 row). P3
        # blocks weave in as soon as the attT columns they read are
        # final: row qq2 finalizes tt8..11, qq1 -> tt4..7, qq0 ->
        # tt0..3, qq3 -> tt12..15 (the tail).
        # Deferred finalize: each group's recip + attT-normalize is
        # emitted only after the NEXT group's first sc matmuls, so the
        # DVE FIFO never stalls at its head waiting for PE to finish the
        # current group's sums (head-of-line blocking that would delay
        # the next group's mask adds and stall its exp->PV chain).
        pending_fin = []

        def run_group(h, qq, fillers=()):
            g = attn_head(h, qq)
            next(g)
            while pending_fin:
                pending_fin.pop(0)()
            for f in fillers:
                f()
            for _ in g:
                pass

        for h in range(HQ):
            run_group(h, 2)
        for h in range(HQ):
            fill = []
            if h >= 1:
                fill.append(lambda: transposes(*pend.pop(0), p1tpB))
            run_group(h, 1, fill)
        tpB_ctx.close()
        for h in range(HQ):
            run_group(h, 0, [lambda hh=h: p3_block(8 + hh)])
        for h in range(HQ):
            run_group(h, 3, [lambda hh=h: p3_block(4 + hh),
                             lambda hh=h: p3_block(hh)])
        while pending_fin:
            pending_fin.pop(0)()
        for tt in range(12, TB):
            p3_block(tt)
        p3_ctx.close()

    nc.compile()
    return nc


def _prep_core_inputs(b, g, x, Wq, bq, Wk, bk, Wv, bv, Wo, bo, qn_w, kn_w,
                      freqs_cos, freqs_sin, mask, np_dt=np.float32):
    f32 = np.float32
    xb = np.ascontiguousarray(x[b], dtype=f32)
    # [tt, csub, cc, tcol]: xt[tt][p][cc*128+tc] = x[b][tt*128+tc][cc*128+p]
    xt = np.ascontiguousarray(
        xb.reshape(TB, 128, CB, 128).transpose(0, 3, 2, 1)
    ).reshape(TB, 128, CB * 128)
    # de-interleave RoPE pairs within each head: head-dim order becomes
    # [0,2,4,...,126, 1,3,...,127]. Applied consistently to Wq and Wk so
    # q.k dot products are unchanged.
    perm = np.concatenate([np.arange(0, D, 2), np.arange(1, D, 2)])
    wq_g = Wq[:, g * 512:(g + 1) * 512].reshape(C, HQ, D)[:, :, perm]
    wq_g = wq_g.reshape(C, 512)
    wk_g = Wk[:, g * 128:(g + 1) * 128][:, perm]
    bq_g = bq[g * 512:(g + 1) * 512].reshape(HQ, D)[:, perm].reshape(512)
    bk_g = bk[g * 128:(g + 1) * 128][perm]
    wcat = np.concatenate([
        wq_g, wk_g, Wv[:, g * 128:(g + 1) * 128],
    ], axis=1).astype(f32)                       # [C, 768]
    # SBUF layout [p, cc, col]: wqkv[p, cc*768+col] = wcat[cc*128+p, col]
    wqkv = np.ascontiguousarray(
        wcat.reshape(CB, 128, 768).transpose(1, 0, 2)).reshape(128, CB * 768)
    bqkv = np.concatenate([
        bq_g, bk_g, bv[g * 128:(g + 1) * 128],
    ]).reshape(1, 768).astype(f32)
    cos = freqs_cos.astype(f32)
    sin = freqs_sin.astype(f32)
    qe, qo = qn_w[0::2].astype(f32), qn_w[1::2].astype(f32)
    ke, ko = kn_w[0::2].astype(f32), kn_w[1::2].astype(f32)
    # tables: [ce, so, se, co] x [5 segs (4 q heads + k)] x 64.
    # norm weights folded in: q segs use qn_w, k seg uses kn_w.
    ce5 = np.stack([cos * qe] * 4 + [cos * ke], axis=1)   # [T, 5, 64]
    so5 = np.stack([sin * qo] * 4 + [sin * ko], axis=1)
    se5 = np.stack([sin * qe] * 4 + [sin * ke], axis=1)
    co5 = np.stack([cos * qo] * 4 + [cos * ko], axis=1)
    tabs = np.stack([ce5, so5, se5, co5], axis=1)          # [T, 4, 5, 64]
    trig = np.ascontiguousarray(tabs.reshape(TB, 128, 4 * 5 * 64), dtype=f32)
    maskt = np.ascontiguousarray(mask[0, 0, :128, :128].T, dtype=f32)
    mask01 = (maskt == 0.0).astype(f32)   # 1 where k<=q (valid), else 0
    cst = np.stack([np.ones((128, 128), f32), np.eye(128, dtype=f32), maskt,
                    mask01])
    # SBUF layout [p, h, c]: wo[p, h*C+c] = Wo[g*512 + h*128 + p, c]
    # (rows of Wo permuted to match the de-interleaved head-dim order of
    # attT... note attT rows are v-dims, NOT rope-permuted: only q/k were
    # permuted; v and Wo keep the original order.)
    wo_t = np.ascontiguousarray(
        Wo[g * 512:(g + 1) * 512].astype(f32).reshape(HQ, 128, C)
        .transpose(1, 0, 2)).reshape(128, HQ * C)
    out = {"xt": xt, "wqkv": wqkv, "bqkv": bqkv, "trig": trig,
           "cst": cst, "wo": wo_t}
    if np_dt is not np.float32:
        for k in ("xt", "wqkv", "bqkv", "trig", "cst", "wo"):
            out[k] = out[k].astype(np_dt)
    return out


def kernel(x, Wq, bq, Wk, bk, Wv, bv, Wo, bo, qn_w, kn_w,
           freqs_cos, freqs_sin, mask, _trace=False, _trace_kwargs=None):
    from concourse.bass_utils import run_bass_kernel_spmd

    args = (np.asarray(x), np.asarray(Wq), np.asarray(bq), np.asarray(Wk),
            np.asarray(bk), np.asarray(Wv), np.asarray(bv), np.asarray(Wo),
            np.asarray(bo), np.asarray(qn_w), np.asarray(kn_w),
            np.asarray(freqs_cos), np.asarray(freqs_sin), np.asarray(mask))
    bo_np = args[8].astype(np.float32)

    dt_name = os.environ.get("BASS_ATTN_DT", "bfloat16")
    with_bias = bool(np.any(args[2]) or np.any(args[4]) or np.any(args[6]))
    key = (dt_name, with_bias)
    if key not in _CACHE:
        _CACHE[key] = _build(dt_name, with_bias)
    nc = _CACHE[key]

    from concourse import mybir
    np_dt = mybir.dt.np(getattr(mybir.dt, dt_name))
    in_maps = [_prep_core_inputs(cid // 4, cid % 4, *args, np_dt=np_dt)
               for cid in range(8)]
    res = run_bass_kernel_spmd(nc, in_maps, core_ids=list(range(8)),
                               trace=_trace, **(_trace_kwargs or {}))
    outs = [res.results[i]["y"].astype(np.float32) for i in range(8)]
    yfull = np.empty((B, T, C), dtype=np.float32)
    for b in range(B):
        yfull[b] = outs[4 * b] + outs[4 * b + 1] + outs[4 * b + 2] + outs[4 * b + 3]
        yfull[b] += bo_np[None, :]
    if _trace:
        kernel._last_result = res
    return yfull


# revision 30
# speedup vs baseline: 1.1578x; 1.1578x over previous
"""LLaMA causal self-attention (GQA) on 8 Trainium2 NeuronCores.

Sharding: 2-way data-parallel over batch x 4-way tensor-parallel over KV
groups. Core cid handles batch b=cid//4 and KV group g=cid%4 (q heads
4g..4g+3, kv head g). Each core computes a partial output y_partial =
att_heads @ Wo_rows; the host sums the 4 partials per batch and adds bo.

Per-core pipeline (all layouts chosen so matmul contraction is on the
partition dim and softmax needs no transposes):
  P1: QKV projection (x^T chunks stationary). RMSNorm stats read the
      PSUM accumulator directly on DVE; rstd via ACT Sqrt + DVE recip +
      one fused-Newton step. RoPE operates on a de-interleaved head
      layout (even dims | odd dims per head, weights permuted host-side)
      so every DVE/Pool op runs on contiguous [128,5,64] APs covering
      all 4 q heads + k in one instruction. PE-transpose q/k to [d, t],
      copy-out on ACT, 3-deep software pipeline.
  P2: attention computed transposed: scoresT[k, q] = kT_j^T @ qT chunks,
      additive causal mask on diagonal blocks, exp without max
      subtraction, softmax denominators via an all-ones stationary
      matmul, PV accumulated in PSUM over j. Emission keeps the PE one
      sc-matmul ahead of the exp it waits on.
  P3: output projection from attT chunks, double-buffered PSUM so the
      PE never stalls (keeps the 2.4 GHz p-state), PSUM -> SBUF copies
      alternate ACT/DVE, DMA out per 128-token block.

Phases are interleaved at emission time: the last three transpose
batches weave between the first attention groups, and the 16 output-
projection blocks weave between later attention groups (each block is
emitted as soon as the attT columns it reads are final), so the PE
queue never drains across phase boundaries.
"""

import os
from contextlib import ExitStack

import numpy as np

B, T, C = 2, 2048, 2048
H, KV = 16, 4
D = 128
HQ = H // KV        # q heads per core = 4
TB = T // 128       # 16
CB = C // 128       # 16
EPS = 1e-5
SCALE = float(np.float32(1.0) / np.sqrt(np.float32(D)))

_CACHE = {}


def _build(dt_name, with_bias=True):
    import concourse.bass as bass
    import concourse.bacc as bacc
    from concourse import mybir
    from concourse.tile import TileContext

    DT = getattr(mybir.dt, dt_name)
    F32 = mybir.dt.float32
    AF = mybir.ActivationFunctionType
    ALU = mybir.AluOpType

    nc = bacc.Bacc(None, target_bir_lowering=False)
    xt = nc.dram_tensor("xt", [TB, 128, CB * 128], DT, kind="ExternalInput")
    wqkv = nc.dram_tensor("wqkv", [128, CB * 768], DT, kind="ExternalInput")
    bqkv = nc.dram_tensor("bqkv", [1, 768], DT, kind="ExternalInput")
    trig = nc.dram_tensor("trig", [TB, 128, 4 * 5 * 64], DT, kind="ExternalInput")
    cst = nc.dram_tensor("cst", [4, 128, 128], DT, kind="ExternalInput")
    wo = nc.dram_tensor("wo", [128, HQ * C], DT, kind="ExternalInput")
    y = nc.dram_tensor("y", [T, C], DT, kind="ExternalOutput")

    with TileContext(nc) as tc, ExitStack() as ctx:
        persist = ctx.enter_context(tc.tile_pool(name="persist", bufs=1))
        # [d, seg, t]: segs 0..3 = q heads, seg 4 = k
        qkT = persist.tile([128, 5, T], DT)
        vbuf = persist.tile([128, TB, 128], DT)   # [t-in-block, j, d]
        attT = persist.tile([128, HQ, T], DT)     # [d, head, t]
        wqkv_sb = persist.tile([128, CB, 768], DT)
        wo_sb = persist.tile([128, HQ, C], DT)
        ones = persist.tile([128, 128], DT)
        ident = persist.tile([128, 128], DT)
        maskt_sb = persist.tile([128, 128], DT)
        bq_sb = persist.tile([1, 768], DT)
        eps_sb = persist.tile([128, 1], F32)

        # ---------------- P1: QKV + RMSNorm + RoPE + transpose ----------
        # P1 pools live in their own stack so their PSUM banks free up
        # before the attention pools are created. p1r (qr tiles) must
        # outlive the last transpose batches woven into the qq0 row.
        p1r = ctx.enter_context(tc.tile_pool(name="p1r", bufs=5))
        p1_ctx = ExitStack()
        p1x = p1_ctx.enter_context(tc.tile_pool(name="p1x", bufs=3))
        p1t = p1_ctx.enter_context(tc.tile_pool(name="p1t", bufs=3))
        p1s = p1_ctx.enter_context(tc.tile_pool(name="p1s", bufs=3))
        p1q = p1_ctx.enter_context(tc.tile_pool(name="p1q", bufs=3))
        p1ps = p1_ctx.enter_context(tc.tile_pool(name="p1ps", bufs=2, space="PSUM"))
        p1tp = p1_ctx.enter_context(tc.tile_pool(name="p1tp", bufs=2, space="PSUM"))

        # startup: stream xt0 in quarters so the first QKV matmuls can
        # begin after ~1/4 of the tile; weave wqkv chunks in between.
        xt0 = p1x.tile([128, CB, 128], DT, tag="xt")
        nc.sync.dma_start(out=xt0[:, 0:4, :], in_=xt[0, :, 0:512])
        nc.sync.dma_start(out=wqkv_sb[:, 0:2, :], in_=wqkv[:, 0:1536])
        nc.sync.dma_start(out=xt0[:, 4:8, :], in_=xt[0, :, 512:1024])
        nc.sync.dma_start(out=wqkv_sb[:, 2:4, :], in_=wqkv[:, 1536:3072])
        nc.sync.dma_start(out=xt0[:, 8:12, :], in_=xt[0, :, 1024:1536])
        nc.sync.dma_start(out=wqkv_sb[:, 4:6, :], in_=wqkv[:, 3072:4608])
        nc.sync.dma_start(out=xt0[:, 12:16, :], in_=xt[0, :, 1536:2048])
        trig0 = p1t.tile([128, 4, 5, 64], DT, tag="trig")
        nc.sync.dma_start(out=trig0, in_=trig[0])
        for wc in range(3, 8):
            nc.sync.dma_start(out=wqkv_sb[:, wc * 2:(wc + 1) * 2, :],
                              in_=wqkv[:, wc * 1536:(wc + 1) * 1536])
        nc.sync.dma_start(out=ones, in_=cst[0])
        nc.sync.dma_start(out=ident, in_=cst[1])
        nc.sync.dma_start(out=maskt_sb, in_=cst[2])
        if with_bias:
            nc.sync.dma_start(out=bq_sb, in_=bqkv[:, :])
        nc.gpsimd.memset(eps_sb, EPS)

        pend = []

        def transposes(qr_prev, ttp, pool):
            # all 5 transposes land in ONE PSUM tile (disjoint 128-col
            # slices), then one wide copy to qkT on ACT
            tp = pool.tile([128, 5, 128], DT, tag="tp")
            for s in range(5):
                nc.tensor.transpose(tp[:, s, :],
                                    qr_prev[:, s * 128:(s + 1) * 128],
                                    ident)
            nc.scalar.activation(
                out=qkT[:, 0:5, ttp * 128:(ttp + 1) * 128], in_=tp,
                func=AF.Copy, scale=1.0, bias=0.0)

        for tt in range(TB):
            if tt == 0:
                xtall, trig_sb = xt0, trig0
            else:
                xtall = p1x.tile([128, CB, 128], DT, tag="xt")
                nc.sync.dma_start(out=xtall, in_=xt[tt])
                trig_sb = p1t.tile([128, 4, 5, 64], DT, tag="trig")
                nc.sync.dma_start(out=trig_sb, in_=trig[tt])
            if tt == 2:
                # wo needed only in P3; keep it off the early DMA path
                nc.sync.dma_start(out=wo_sb, in_=wo[:, :])

            qkv_ps = p1ps.tile([128, 768], F32, tag="qkv")
            last = CB - 1
            for cc in range(CB):
                stop_here = (not with_bias) and cc == last
                nc.tensor.matmul(qkv_ps[:, 0:512], xtall[:, cc, :],
                                 wqkv_sb[:, cc, 0:512],
                                 start=(cc == 0), stop=stop_here)
                nc.tensor.matmul(qkv_ps[:, 512:768], xtall[:, cc, :],
                                 wqkv_sb[:, cc, 512:768],
                                 start=(cc == 0), stop=stop_here)
            if with_bias:
                nc.tensor.matmul(qkv_ps[:, 0:512], ones[0:1, :],
                                 bq_sb[0:1, 0:512], start=False, stop=True)
                nc.tensor.matmul(qkv_ps[:, 512:768], ones[0:1, :],
                                 bq_sb[0:1, 512:768], start=False,
                                 stop=True)

            # software pipeline: transpose qr from THREE tiles back
            if len(pend) == 3:
                transposes(*pend.pop(0), p1tp)

            # RMSNorm stats: squares on ACT (single PSUM read), segmented
            # reduce + rstd + normalize on DVE -- except for the LAST
            # three tts, where the chain runs on ACT/Pool instead so the
            # DVE queue is empty when the first attention groups need
            # their causal-mask adds (DVE is the only engine that can
            # add the mask into the PSUM score tiles).
            # tt14/15 chains run on Pool: their transposes are woven late
            # into the second attention row, and keeping these chains off
            # DVE frees it for the first rows' causal-mask adds.
            tail = tt >= TB - 2
            ve = nc.gpsimd if tail else nc.vector
            sqs = p1s.tile([128, 640], F32, tag="sqs")
            ssq = p1s.tile([128, 8], F32, tag="ssq")
            nc.scalar.activation(out=sqs, in_=qkv_ps[:, 0:640],
                                 func=AF.Square, scale=1.0, bias=0.0)
            nc.vector.tensor_reduce(
                out=ssq[:, 0:5],
                in_=sqs.rearrange("p (s d) -> p s d", s=5, d=128),
                axis=mybir.AxisListType.X, op=ALU.add)
            # rstd = rsqrt(ssq/D + EPS): ACT Sqrt + DVE recip, then one
            # fused Newton step rstd = r0 * (1.5 - (0.5/D)*ssq*r0^2)
            sq5 = p1s.tile([128, 8], F32, tag="sq5")
            nc.scalar.activation(out=sq5[:, 0:5], in_=ssq[:, 0:5],
                                 func=AF.Sqrt, scale=1.0 / D,
                                 bias=eps_sb[:, 0:1])
            rstd = p1s.tile([128, 8], F32, tag="rstd")
            nc.vector.reciprocal_approx_fast(out=rstd[:, 0:5], in_=sq5[:, 0:5])

            # normalize q+k in ONE op: rstd broadcast over each seg.
            # Tail tts: ACT copies PSUM->SBUF first, Pool normalizes.
            qn = p1q.tile([128, 640], F32, tag="qn")
            rstd_b = bass.AP(tensor=rstd.tensor, offset=rstd.offset,
                             ap=[list(rstd.ap[0]), [1, 5], [0, 128]])
            if tail:
                qs = p1q.tile([128, 640], F32, tag="qs")
                nc.scalar.activation(out=qs, in_=qkv_ps[:, 0:640],
                                     func=AF.Copy, scale=1.0, bias=0.0)
                qsrc = qs
            else:
                qsrc = qkv_ps[:, 0:640]
            ve.tensor_mul(
                qn.rearrange("p (s d) -> p s d", s=5, d=128),
                qsrc.rearrange("p (s d) -> p s d", s=5, d=128),
                rstd_b)
            # v copy on ACT (PSUM -> SBUF bf16)
            nc.scalar.activation(out=vbuf[:, tt, :], in_=qkv_ps[:, 640:768],
                                 func=AF.Copy, scale=1.0, bias=0.0)

            # RoPE on de-interleaved layout: evens at [s*128, s*128+64),
            # odds at +64. One [128,5,64] op covers all 4 q heads + k.
            qr = p1r.tile([128, 640], DT, tag="qr")
            qn3 = qn.rearrange("p (s d) -> p s d", s=5, d=128)
            qr3 = qr.rearrange("p (s d) -> p s d", s=5, d=128)
            qe, qo = qn3[:, :, 0:64], qn3[:, :, 64:128]
            re, ro = qr3[:, :, 0:64], qr3[:, :, 64:128]
            ce, so = trig_sb[:, 0], trig_sb[:, 1]
            se, co = trig_sb[:, 2], trig_sb[:, 3]
            ta = p1s.tile([128, 5, 64], F32, tag="ra")
            tb = p1s.tile([128, 5, 64], F32, tag="rb")
            ve.tensor_mul(ta, qe, ce)
            nc.gpsimd.tensor_mul(tb, qo, so)
            ve.tensor_sub(re, ta, tb)
            tc_ = p1s.tile([128, 5, 64], F32, tag="rc")
            td = p1s.tile([128, 5, 64], F32, tag="rd")
            nc.gpsimd.tensor_mul(tc_, qe, se)
            ve.tensor_mul(td, qo, co)
            nc.gpsimd.tensor_add(ro, tc_, td)
            pend.append((qr, tt))

        # ------- P2+P3 interleaved: attention + output projection -------
        # PSUM budget during attn: sc 2 + outT 2 + sums 2 + (tpB 1 then
        # y 2) = 7-8 banks.
        p1_ctx.close()
        p2 = ctx.enter_context(tc.tile_pool(name="p2", bufs=6))
        p2a = ctx.enter_context(tc.tile_pool(name="p2a", bufs=6))
        p2v = ctx.enter_context(tc.tile_pool(name="p2v", bufs=2))
        p2o = ctx.enter_context(tc.tile_pool(name="p2o", bufs=2, space="PSUM"))
        p2s = ctx.enter_context(tc.tile_pool(name="p2s", bufs=2, space="PSUM"))
        p2sc = ctx.enter_context(tc.tile_pool(name="p2sc", bufs=2, space="PSUM"))
        tpB_ctx = ExitStack()
        p1tpB = tpB_ctx.enter_context(
            tc.tile_pool(name="p1tpB", bufs=1, space="PSUM"))

        def attn_head(h, qq):
            """Emit one attention group. Yields once after the first two
            sc matmuls so callers can weave PE filler work in."""
            q0 = qq * 512
            jmax = qq * 4 + 3
            outT = p2o.tile([128, 512], F32, tag="outT")
            sums = p2s.tile([128, 512], F32, tag="sums")
            scs = {}

            def emit_sc(j):
                qlo = max(q0, j * 128)
                ob = qlo - q0
                sc = p2sc.tile([128, 512], F32, tag="sc")
                nc.tensor.matmul(sc[:, ob:512], qkT[:, 4, j * 128:(j + 1) * 128],
                                 qkT[:, h, qlo:q0 + 512],
                                 start=True, stop=True)
                if j * 128 >= q0:  # diagonal: causal mask
                    nc.vector.tensor_add(sc[:, ob:ob + 128],
                                         sc[:, ob:ob + 128], maskt_sb)
                scs[j] = (sc, ob)

            emit_sc(0)
            emit_sc(1)
            yield
            for j in range(jmax + 1):
                sc, ob = scs.pop(j)
                pT = p2.tile([128, 512], DT, tag="pT")
                nc.scalar.activation(out=pT[:, ob:512], in_=sc[:, ob:512],
                                     func=AF.Exp, scale=SCALE)
                nc.tensor.matmul(outT[:, ob:512], vbuf[:, j, :],
                                 pT[:, ob:512],
                                 start=(j == 0), stop=(j == jmax),
                                 skip_group_check=True)
                nc.tensor.matmul(sums[:, ob:512], ones,
                                 pT[:, ob:512],
                                 start=(j == 0), stop=(j == jmax),
                                 skip_group_check=True)
                if j + 2 <= jmax:
                    emit_sc(j + 2)

            def finalize():
                inv = p2v.tile([128, 512], F32, tag="inv")
                nc.vector.reciprocal_approx_fast(out=inv, in_=sums)
                nc.vector.tensor_mul(attT[:, h, q0:q0 + 512], outT, inv)

            pending_fin.append(finalize)

        p3_ctx = ExitStack()
        p3sb = None
        p3ps = None

        def p3_block(tt):
            nonlocal p3sb, p3ps
            if p3ps is None:
                p3sb = p3_ctx.enter_context(tc.tile_pool(name="p3sb", bufs=2))
                p3ps = p3_ctx.enter_context(
                    tc.tile_pool(name="p3ps", bufs=2, space="PSUM"))
            y_sb = p3sb.tile([128, 2048], DT, tag="ysb")
            for c4 in range(4):
                y_ps = p3ps.tile([128, 512], F32, tag="y")
                for hh in range(HQ):
                    nc.tensor.matmul(y_ps,
                                     attT[:, hh, tt * 128:(tt + 1) * 128],
                                     wo_sb[:, hh, c4 * 512:(c4 + 1) * 512],
                                     start=(hh == 0), stop=(hh == HQ - 1))
                dst = y_sb[:, c4 * 512:(c4 + 1) * 512]
                if c4 % 2 == 0:
                    nc.scalar.activation(out=dst, in_=y_ps, func=AF.Copy,
                                         scale=1.0, bias=0.0)
                else:
                    nc.vector.tensor_copy(dst, y_ps)
            nc.sync.dma_start(out=y[tt * 128:(tt + 1) * 128, :], in_=y_sb)

        # Row order [qq2, qq1, qq0, qq3]: the long, DVE-light qq2 row
        # runs first while DVE drains the P1 tail chains (the early rows
        # are short and DVE-heavy: masks + recip + attT-normalize nearly
        # saturate DVE). The last three transpose batches weave into the
        # qq1 row (their outputs are only read by the qq3 row). P3
        # blocks weave in as soon as the attT columns they read are
        # final: row qq2 finalizes tt8..11, qq1 -> tt4..7, qq0 ->
        # tt0..3, qq3 -> tt12..15 (the tail).
        # Deferred finalize: each group's recip + attT-normalize is
        # emitted only after the NEXT group's first sc matmuls, so the
        # DVE FIFO never stalls at its head waiting for PE to finish the
        # current group's sums (head-of-line blocking that would delay
        # the next group's mask adds and stall its exp->PV chain).
        pending_fin = []

        def run_group(h, qq, fillers=()):
            g = attn_head(h, qq)
            next(g)
            while pending_fin:
                pending_fin.pop(0)()
            for f in fillers:
                f()
            for _ in g:
                pass

        for h in range(HQ):
            run_group(h, 2)
        for h in range(HQ):
            fill = []
            if h >= 1:
                fill.append(lambda: transposes(*pend.pop(0), p1tpB))
            run_group(h, 1, fill)
        tpB_ctx.close()
        for h in range(HQ):
            run_group(h, 0, [lambda hh=h: p3_block(8 + hh)])
        for h in range(HQ):
            run_group(h, 3, [lambda hh=h: p3_block(4 + hh),
                             lambda hh=h: p3_block(hh)])
        while pending_fin:
            pending_fin.pop(0)()
        for tt in range(12, TB):
            p3_block(tt)
        p3_ctx.close()

    nc.compile()
    return nc


def _prep_core_inputs(b, g, x, Wq, bq, Wk, bk, Wv, bv, Wo, bo, qn_w, kn_w,
                      freqs_cos, freqs_sin, mask, np_dt=np.float32):
    f32 = np.float32
    xb = np.ascontiguousarray(x[b], dtype=f32)
    # [tt, csub, cc, tcol]: xt[tt][p][cc*128+tc] = x[b][tt*128+tc][cc*128+p]
    xt = np.ascontiguousarray(
        xb.reshape(TB, 128, CB, 128).transpose(0, 3, 2, 1)
    ).reshape(TB, 128, CB * 128)
    # de-interleave RoPE pairs within each head: head-dim order becomes
    # [0,2,4,...,126, 1,3,...,127]. Applied consistently to Wq and Wk so
    # q.k dot products are unchanged.
    perm = np.concatenate([np.arange(0, D, 2), np.arange(1, D, 2)])
    wq_g = Wq[:, g * 512:(g + 1) * 512].reshape(C, HQ, D)[:, :, perm]
    wq_g = wq_g.reshape(C, 512)
    wk_g = Wk[:, g * 128:(g + 1) * 128][:, perm]
    bq_g = bq[g * 512:(g + 1) * 512].reshape(HQ, D)[:, perm].reshape(512)
    bk_g = bk[g * 128:(g + 1) * 128][perm]
    wcat = np.concatenate([
        wq_g, wk_g, Wv[:, g * 128:(g + 1) * 128],
    ], axis=1).astype(f32)                       # [C, 768]
    # SBUF layout [p, cc, col]: wqkv[p, cc*768+col] = wcat[cc*128+p, col]
    wqkv = np.ascontiguousarray(
        wcat.reshape(CB, 128, 768).transpose(1, 0, 2)).reshape(128, CB * 768)
    bqkv = np.concatenate([
        bq_g, bk_g, bv[g * 128:(g + 1) * 128],
    ]).reshape(1, 768).astype(f32)
    cos = freqs_cos.astype(f32)
    sin = freqs_sin.astype(f32)
    qe, qo = qn_w[0::2].astype(f32), qn_w[1::2].astype(f32)
    ke, ko = kn_w[0::2].astype(f32), kn_w[1::2].astype(f32)
    # tables: [ce, so, se, co] x [5 segs (4 q heads + k)] x 64.
    # norm weights folded in: q segs use qn_w, k seg uses kn_w.
    ce5 = np.stack([cos * qe] * 4 + [cos * ke], axis=1)   # [T, 5, 64]
    so5 = np.stack([sin * qo] * 4 + [sin * ko], axis=1)
    se5 = np.stack([sin * qe] * 4 + [sin * ke], axis=1)
    co5 = np.stack([cos * qo] * 4 + [cos * ko], axis=1)
    tabs = np.stack([ce5, so5, se5, co5], axis=1)          # [T, 4, 5, 64]
    trig = np.ascontiguousarray(tabs.reshape(TB, 128, 4 * 5 * 64), dtype=f32)
    maskt = np.ascontiguousarray(mask[0, 0, :128, :128].T, dtype=f32)
    mask01 = (maskt == 0.0).astype(f32)   # 1 where k<=q (valid), else 0
    cst = np.stack([np.ones((128, 128), f32), np.eye(128, dtype=f32), maskt,
                    mask01])
    # SBUF layout [p, h, c]: wo[p, h*C+c] = Wo[g*512 + h*128 + p, c]
    # (rows of Wo permuted to match the de-interleaved head-dim order of
    # attT... note attT rows are v-dims, NOT rope-permuted: only q/k were
    # permuted; v and Wo keep the original order.)
    wo_t = np.ascontiguousarray(
        Wo[g * 512:(g + 1) * 512].astype(f32).reshape(HQ, 128, C)
        .transpose(1, 0, 2)).reshape(128, HQ * C)
    out = {"xt": xt, "wqkv": wqkv, "bqkv": bqkv, "trig": trig,
           "cst": cst, "wo": wo_t}
    if np_dt is not np.float32:
        for k in ("xt", "wqkv", "bqkv", "trig", "cst", "wo"):
            out[k] = out[k].astype(np_dt)
    return out


def kernel(x, Wq, bq, Wk, bk, Wv, bv, Wo, bo, qn_w, kn_w,
           freqs_cos, freqs_sin, mask, _trace=False, _trace_kwargs=None):
    from concourse.bass_utils import run_bass_kernel_spmd

    args = (np.asarray(x), np.asarray(Wq), np.asarray(bq), np.asarray(Wk),
            np.asarray(bk), np.asarray(Wv), np.asarray(bv), np.asarray(Wo),
            np.asarray(bo), np.asarray(qn_w), np.asarray(kn_w),
            np.asarray(freqs_cos), np.asarray(freqs_sin), np.asarray(mask))
    bo_np = args[8].astype(np.float32)

    dt_name = os.environ.get("BASS_ATTN_DT", "bfloat16")
    with_bias = bool(np.any(args[2]) or np.any(args[4]) or np.any(args[6]))
    key = (dt_name, with_bias)
    if key not in _CACHE:
        _CACHE[key] = _build(dt_name, with_bias)
    nc = _CACHE[key]

    from concourse import mybir
    np_dt = mybir.dt.np(getattr(mybir.dt, dt_name))
    in_maps = [_prep_core_inputs(cid // 4, cid % 4, *args, np_dt=np_dt)
               for cid in range(8)]
    res = run_bass_kernel_spmd(nc, in_maps, core_ids=list(range(8)),
                               trace=_trace, **(_trace_kwargs or {}))
    outs = [res.results[i]["y"].astype(np.float32) for i in range(8)]
    yfull = np.empty((B, T, C), dtype=np.float32)
    for b in range(B):
        yfull[b] = outs[4 * b] + outs[4 * b + 1] + outs[4 * b + 2] + outs[4 * b + 3]
        yfull[b] += bo_np[None, :]
    if _trace:
        kernel._last_result = res
    return yfull
